# revision 20
# baseline (speedup 1.0000x reference)
"""Trainium2 Bass kernel for nn_BiLSTM_58351425683854 — fp8 DoubleRow version.

Math (see baseline kernel.py for derivation): output depends only on hf/hb
from the contractive interaction fixed point; 2 steps reproduce the 100-step
reference far below the 2e-2 gate. Precision schedule per dense
[x1, hb2, hf2, x2, x1b, hb', hf']:
  '8' = fp8e4m3 weights + DoubleRow matmuls (2 k-tiles/instr); multi-tensor
        rhs summed by DVE (fp8+fp8 -> fp8); sigma stored fp8.
  'A' = like '8' but the two rhs tensors are PSUM-accumulated (W@a + W@b)
        instead of DVE-added — 2x the DR matmuls but no DVE stage on the
        chain.
  'H' = hybrid of the two: kp0 reads a DVE-added pair (those adds hide
        under the predecessor's sigma serialization), kp1 PSUM-accumulates
        (the chain-critical tail stays add-free). 12 matmuls per dense vs
        16 for 'A' / 8 for '8'; used for every on-chain two-input dense
        (hf2 stays '8': fully off-chain).
  'F' = like '8' but sigma stored fp32.  'b' = bf16 weights, bf16 DVE-add
        rhs, fp32 sigma (slower; kept for a higher-accuracy fallback).
Final step's hb'/hf' are always stored fp32 regardless of code.
HW rel err: ("888HHHH","HH8HHbb") -> 4.10e-3 (absmax-rel 1.32e-2), ~55us.

Emission order runs the critical chain first (x1 -> hb2 -> x2 -> x1b ->
hb/hf; hf2 is off-chain and fills engine gaps). The last two denses are
fused so each m-tile's output add + DMA chases its sigmas.

DMA notes: DRAM->SBUF runs ~48GB/s per queue with 2KB/partition packets.
Only SP/gpsimd/ACT can issue DMAs; ACT must stay free for sigmas, so it
only issues first-wave chunks (bias, x8, W1) before its first sigma.

Sharding: rows of (seq*batch, H) split across 8 cores, weights replicated,
activations feature-major (H on partitions); no cross-core communication.
"""

import numpy as np
import ml_dtypes

import concourse.bass as bass
import concourse.bacc as bacc
import concourse.mybir as mybir
import concourse.tile as tile
from concourse.bass_utils import run_bass_kernel_spmd

SEQ, B, H = 100, 30, 512
N_CORES = 8
ROWS = SEQ * B // N_CORES   # 375
ROWSP = ROWS + 1            # 376, even for DR pairing
PAIR = 2 * ROWSP            # 752
KT = H // 128
MT = H // 128
F32 = mybir.dt.float32
F8 = mybir.dt.float8e4
BF16 = mybir.dt.bfloat16
SIG = mybir.ActivationFunctionType.Sigmoid
DR = mybir.MatmulPerfMode.DoubleRow

DEFAULT_STEPS = ("888HHHH", "HH8HHbb")
DENSE_W = (0, 1, 2, 3, 0, 1, 2)  # weight index per dense slot


def _b_ws(steps):
    return sorted({DENSE_W[i] for st in steps for i, c in enumerate(st)
                   if c == "b"})


def build_program(steps=DEFAULT_STEPS):
    nc = bacc.Bacc("TRN2", target_bir_lowering=False)

    b_ws = _b_ws(steps)
    x8_d = nc.declare_dram_parameter("x8", [128, KT * ROWSP], F8, isOutput=False)
    w8_d = nc.declare_dram_parameter("w8", [128, 4 * 2048], F8, isOutput=False)
    wb_d = (nc.declare_dram_parameter("wb", [len(b_ws), 128, KT * H], BF16,
                                      isOutput=False) if b_ws else None)
    b_d = nc.declare_dram_parameter("bias", [128, 16], F32, isOutput=False)
    out_d = nc.declare_dram_parameter("out", [H, ROWSP], F32, isOutput=True)

    with tile.TileContext(nc) as tc:
        with (
            tc.tile_pool(name="consts", bufs=1) as cpool,
            tc.tile_pool(name="acts", bufs=2) as apool,
            tc.tile_pool(name="tmps", bufs=1) as tpool,
            tc.tile_pool(name="psum", bufs=2, space=bass.MemorySpace.PSUM) as pspool,
        ):
            w8_slab = cpool.tile([128, 4 * 2048], F8, name="w8_slab")
            wb_slab = (cpool.tile([128, len(b_ws) * KT * H], BF16,
                                  name="wb_slab") if b_ws else None)
            bias_slab = cpool.tile([128, 16], F32, name="bias_slab")
            x8_slab = cpool.tile([128, KT * ROWSP], F8, name="x8_slab")

            # ---- input DMAs ----
            # First wave on all 3 queues: bias, W1 thirds, x8 thirds.
            def spread(dst_slab, src_2d, col0, cols, engines):
                n = len(engines)
                q = (cols + n - 1) // n
                for i, eng in enumerate(engines):
                    a, b2 = col0 + i * q, col0 + min((i + 1) * q, cols)
                    if a >= b2:
                        continue
                    eng.dma_start(dst_slab[:, a:b2], src_2d[:, a:b2])

            # First wave kp-aligned so x1's kp0 matmuls gate on exactly the
            # ranges they read: W1-kp0 | x8-kp0 land first, kp1 chunks chase.
            # (Finer splits measured WORSE: each extra dma_start costs
            # ~0.65us of issue time on its engine.)
            nc.sync.dma_start(w8_slab[:, 0:1024], w8_d[:, 0:1024])
            nc.gpsimd.dma_start(x8_slab[:, 0:PAIR], x8_d[:, 0:PAIR])
            nc.scalar.dma_start(w8_slab[:, 1024:2048], w8_d[:, 1024:2048])
            nc.gpsimd.dma_start(x8_slab[:, PAIR:2 * PAIR],
                                x8_d[:, PAIR:2 * PAIR])
            nc.scalar.dma_start(bias_slab[:], b_d[:])
            # Rest in dense-use order (W2, W4, W3), then the bf16 tail
            # weights. W2/W3 second halves ride the scalar queue (idle after
            # its first-wave issues, ~1.3us, done well before the first
            # sigma): measured 2.15us of hb2 stall when W2-kp1 queued third
            # on gpsimd behind both x8 halves.
            nc.sync.dma_start(w8_slab[:, 2048:3072], w8_d[:, 2048:3072])
            nc.scalar.dma_start(w8_slab[:, 3072:4096], w8_d[:, 3072:4096])
            engs2 = [nc.sync, nc.gpsimd]
            spread(w8_slab, w8_d, 3 * 2048, 2048, engs2)   # W4
            nc.gpsimd.dma_start(w8_slab[:, 4096:5120], w8_d[:, 4096:5120])
            nc.scalar.dma_start(w8_slab[:, 5120:6144], w8_d[:, 5120:6144])
            if b_ws:
                for i in range(len(b_ws)):
                    q = KT * H // 2
                    for j in range(2):
                        engs2[j].dma_start(
                            wb_slab[:, i * KT * H + j * q:
                                    i * KT * H + (j + 1) * q],
                            wb_d[i][:, j * q:(j + 1) * q])

            bt = [[bias_slab[:, w * MT + m: w * MT + m + 1] for m in range(MT)]
                  for w in range(4)]

            def w8v(w, kp, m):
                ofs = w * 2048 + (kp * 4 + m) * 256
                return w8_slab[:, ofs:ofs + 256].rearrange(
                    "p (two f) -> p two f", two=2)

            def wbv(w, k, m):
                i = b_ws.index(w)
                ofs = i * 2048 + k * 512 + m * 128
                return wb_slab[:, ofs:ofs + 128]

            def kp_view(slab, kp):
                return slab[:, kp * PAIR:(kp + 1) * PAIR].rearrange(
                    "p (two n) -> p two n", two=2)

            def mkrhs(inputs, dtype, tag):
                """rhs slab = sum(inputs), k-granular DVE adds."""
                if len(inputs) == 1 and inputs[0].dtype == dtype:
                    return inputs[0]
                out = tpool.tile([128, KT * ROWSP], dtype, tag=tag, name=tag)
                for k in range(KT):
                    sl = slice(k * ROWSP, (k + 1) * ROWSP)
                    if len(inputs) == 1:
                        nc.vector.tensor_copy(out[:, sl], inputs[0][:, sl])
                    else:
                        nc.vector.tensor_add(out[:, sl], inputs[0][:, sl],
                                             inputs[1][:, sl])
                return out

            # ---- per-m-tile emitters ----
            def mm_sig(code, w, rhs, m, ps, out, obt):
                """matmuls for m-tile m into ps, then sigma into out.
                'A' takes a LIST of fp8 slabs PSUM-accumulated (no DVE add
                on the chain); '8'/'F' take a pre-summed fp8 slab."""
                if code == "A":
                    n = 2 * len(rhs)
                    i = 0
                    for kp in range(2):
                        for inp in rhs:
                            nc.tensor.matmul(ps[:, :ROWSP], w8v(w, kp, m),
                                             kp_view(inp, kp),
                                             start=(i == 0),
                                             stop=(i == n - 1),
                                             perf_mode=DR)
                            i += 1
                elif code == "H":
                    # hybrid: kp0 from the DVE-added pair (hides under the
                    # predecessor's sigma serialization), kp1 PSUM-accum
                    # (chain-critical, no add latency). inputs are ordered
                    # (early, late): the early input's kp1 matmul goes first
                    # so it issues before the late input's sigmas finish.
                    rhs0, inputs = rhs
                    nc.tensor.matmul(ps[:, :ROWSP], w8v(w, 1, m),
                                     kp_view(inputs[0], 1),
                                     start=True, stop=False, perf_mode=DR)
                    nc.tensor.matmul(ps[:, :ROWSP], w8v(w, 0, m),
                                     rhs0[:].rearrange("p (two n) -> p two n",
                                                       two=2),
                                     start=False, stop=False, perf_mode=DR)
                    nc.tensor.matmul(ps[:, :ROWSP], w8v(w, 1, m),
                                     kp_view(inputs[1], 1), start=False,
                                     stop=True, perf_mode=DR)
                elif code in ("8", "F"):
                    for kp in range(2):
                        nc.tensor.matmul(ps[:, :ROWSP], w8v(w, kp, m),
                                         kp_view(rhs, kp),
                                         start=(kp == 0), stop=(kp == 1),
                                         perf_mode=DR)
                else:
                    for k in range(KT):
                        nc.tensor.matmul(
                            ps[:, :ROWSP], wbv(w, k, m),
                            rhs[:, k * ROWSP:(k + 1) * ROWSP],
                            start=(k == 0), stop=(k == KT - 1))
                nc.scalar.activation(out[:, m * ROWSP:(m + 1) * ROWSP],
                                     ps[:, :ROWSP], SIG, bias=obt[m])

            def dense(code, inputs, slot, tag, out_dtype=None, bufs=1):
                w = DENSE_W[slot]
                if code in ("A", "H"):
                    for inp in inputs:
                        assert inp.dtype == F8, \
                            f"fp8 dense {tag} needs fp8 inputs"
                    if len(inputs) == 1:
                        code, rhs = "8", inputs[0]
                    elif code == "H":
                        rhs0 = tpool.tile([128, PAIR], F8, tag="rh_" + tag,
                                          name="rh_" + tag)
                        for k in range(2):
                            sl = slice(k * ROWSP, (k + 1) * ROWSP)
                            nc.vector.tensor_add(rhs0[:, sl],
                                                 inputs[0][:, sl],
                                                 inputs[1][:, sl])
                        rhs = (rhs0, inputs)
                    else:
                        rhs = inputs
                    odt = out_dtype or F8
                elif code in ("8", "F"):
                    for inp in inputs:
                        assert inp.dtype == F8, \
                            f"fp8 dense {tag} needs fp8 inputs"
                    rhs = mkrhs(inputs, F8, "r8_" + tag)
                    odt = out_dtype or (F32 if code == "F" else F8)
                else:
                    rhs = mkrhs(inputs, BF16, "rb_" + tag)
                    odt = out_dtype or F32
                out = apool.tile([128, KT * ROWSP], odt, tag=tag + code,
                                 name=tag + code, bufs=bufs)
                for m in range(MT):
                    ps = pspool.tile([128, 512], F32, tag=f"ps{m}",
                                     name=f"ps_{tag}{m}")
                    mm_sig(code, w, rhs, m, ps, out, bt[w])
                return out

            def fused_tail(dhb, dhf, hb2, x1b, hf2, out_slab):
                """Last two denses, interleaved per m-tile with the output
                add + DMA chasing each m's sigmas."""
                dhb = "A" if dhb == "H" else dhb
                dhf = "A" if dhf == "H" else dhf
                if dhb == "A":
                    rhb, rhf = [hb2, x1b], [x1b, hf2]
                else:
                    rhb = mkrhs([hb2, x1b],
                                F8 if dhb in ("8", "F") else BF16, "r_thb")
                    rhf = mkrhs([x1b, hf2],
                                F8 if dhf in ("8", "F") else BF16, "r_thf")
                ohb = apool.tile([128, KT * ROWSP], F32, tag="thb",
                                 name="thb")
                ohf = apool.tile([128, KT * ROWSP], F32, tag="thf",
                                 name="thf")
                for m in range(MT):
                    sl = slice(m * ROWSP, (m + 1) * ROWSP)
                    ps1 = pspool.tile([128, 512], F32, tag=f"ps{m}",
                                      name=f"ps_thb{m}")
                    mm_sig(dhb, 1, rhb, m, ps1, ohb, bt[1])
                    ps2 = pspool.tile([128, 512], F32,
                                      tag=f"ps{(m + 1) % MT}",
                                      name=f"ps_thf{m}")
                    mm_sig(dhf, 2, rhf, m, ps2, ohf, bt[2])
                    nc.vector.tensor_add(out_slab[:, sl], ohb[:, sl],
                                         ohf[:, sl])
                    eng = nc.sync if m % 2 == 0 else nc.gpsimd
                    eng.dma_start(
                        out_d.rearrange("(k p) n -> k p n", p=128)[m],
                        out_slab[:, sl])

            # ---- fixed-point steps; chain-first emission ----
            out_slab = cpool.tile([128, KT * ROWSP], F32, name="out_slab")
            hf = hb = None
            for s, d in enumerate(steps):
                assert len(d) == 7 and set(d) <= {"8", "b", "F", "A", "H"}
                last = s == len(steps) - 1
                if hf is None:
                    x1 = dense(d[0], [x8_slab], 0, "x1")
                    hb2 = dense(d[1], [x1], 1, "hb2")
                    # hf2 before x2: hf2 is gated only by x1, so its sigmas
                    # fill the hb2->x2 gap and stop being x1b's late gate.
                    hf2 = dense(d[2], [x1], 2, "hf2")
                    x2 = dense(d[3], [x1, hb2], 3, "x2")
                else:
                    x1 = dense(d[0], [x8_slab, hf], 0, "x1")
                    hb2 = dense(d[1], [hb, x1], 1, "hb2")
                    hf2 = dense(d[2], [x1, hf], 2, "hf2")
                    x2 = dense(d[3], [x1, hb2], 3, "x2")
                x1b = dense(d[4], [hf2, x2], 4, "x1b")
                if last:
                    fused_tail(d[5], d[6], hb2, x1b, hf2, out_slab)
                else:
                    # hf first: its sigmas gate the next step's x1/hf2,
                    # while hb is only needed one hop later (hb2').
                    hf = dense(d[6], [hf2, x1b], 6, "hfc", bufs=2)
                    hb = dense(d[5], [hb2, x1b], 5, "hbc", bufs=2)

    nc.compile()
    return nc


_PROGRAM_CACHE = {}


def _get_program(steps):
    key = tuple(steps)
    if key not in _PROGRAM_CACHE:
        _PROGRAM_CACHE[key] = build_program(key)
    return _PROGRAM_CACHE[key]


def _prep_host(inputs, steps):
    inp = {k: np.asarray(v) for k, v in inputs.items()}
    X = np.ascontiguousarray(
        inp["inputs"].astype(np.float32).reshape(SEQ * B, H))
    Wt = [np.ascontiguousarray(inp[f"W{i}"].astype(np.float32).T)
          for i in (1, 2, 3, 4)]

    # fp8 weights in DoubleRow layout: [p, w, kp, m, t(2), j(128)]
    w8 = np.zeros((128, 4, 2, 4, 2, 128), ml_dtypes.float8_e4m3)
    for w in range(4):
        W8 = Wt[w].astype(ml_dtypes.float8_e4m3)
        for kp in range(2):
            for m in range(4):
                for t in range(2):
                    k = 2 * kp + t
                    w8[:, w, kp, m, t, :] = \
                        W8[k * 128:(k + 1) * 128, m * 128:(m + 1) * 128]
    w8 = np.ascontiguousarray(w8.reshape(128, 4 * 2048))

    b_ws = _b_ws(steps)
    wb = None
    if b_ws:
        wb = np.zeros((len(b_ws), 128, KT * H), ml_dtypes.bfloat16)
        for i, w in enumerate(b_ws):
            Wb = Wt[w].astype(ml_dtypes.bfloat16)
            for k in range(KT):
                wb[i][:, k * 512:(k + 1) * 512] = Wb[k * 128:(k + 1) * 128, :]
        wb = np.ascontiguousarray(wb)
    # bias pre-arranged into slab layout [p, w*4+m] = b_w[m*128+p]
    Bv = np.zeros((128, 16), np.float32)
    for w in range(4):
        bw = inp[f"b{w + 1}"].astype(np.float32)
        for m in range(4):
            Bv[:, w * 4 + m] = bw[m * 128:(m + 1) * 128]
    return X, w8, wb, np.ascontiguousarray(Bv)


def run(inputs, steps=DEFAULT_STEPS, trace=False):
    X, w8, wb, Bv = _prep_host(inputs, steps)
    nc = _get_program(steps)
    in_maps = []
    for c in range(N_CORES):
        xT = np.zeros((H, ROWSP), np.float32)
        xT[:, :ROWS] = X[c * ROWS:(c + 1) * ROWS].T
        x8 = np.zeros((128, KT * ROWSP), ml_dtypes.float8_e4m3)
        for k in range(KT):
            x8[:, k * ROWSP:(k + 1) * ROWSP] = \
                xT[k * 128:(k + 1) * 128].astype(ml_dtypes.float8_e4m3)
        m = {"x8": x8, "w8": w8, "bias": Bv}
        if wb is not None:
            m["wb"] = wb
        in_maps.append(m)
    res = run_bass_kernel_spmd(nc, in_maps, list(range(N_CORES)), trace=trace)
    outT = np.concatenate(
        [res.results[c]["out"][:, :ROWS] for c in range(N_CORES)], axis=1)
    full = (np.ascontiguousarray(outT.T) * np.float32(0.5)).reshape(SEQ, B, H)
    full = full.astype(np.float32)
    return (full, res) if trace else (full, None)


def kernel(**inputs):
    full, _ = run(inputs)
    return full
